# revision 10
# baseline (speedup 1.0000x reference)
"""Trainium2 Bass kernel for nn_MAMLAwareGANLoss.

Reference computation (B=1024, Z=256, H=W=128, N=H*W=16384):
    fake   = tanh(noise @ Wg)                      # [B, N]
    d_fake = fake @ Wd                             # [B, 1]
    g_loss = mean(softplus(-d_fake))               # (+ 0.0 * sum(d_real) == 0)
    solvability_loss = mean(per-sample flood-fill penalty of (fake == 1.0) walls)
    cur    = mean(fake == 1.0)
    difficulty_loss  = (cur - current_difficulty)^2
    loss   = g_loss + w_s * solvability_loss + w_d * difficulty_loss

Key structural facts used here:
  * real_mazes enters only through `0.0 * sum(d_real)` == exactly 0.0 -> never loaded.
  * "walls" are cells where float32 tanh(x) rounds to exactly 1.0, which requires
    x >= ~9.01.  We prove on the host (Cauchy-Schwarz over the actual inputs:
    max_b ||noise_b|| * max_n ||Wg[:, n]||) that no |x| can exceed the threshold,
    hence wall count == 0 exactly => solvability_loss == 0.0 and cur == 0.0.
    If the bound ever fails we fall back to an exact host recomputation.
  * Therefore the device only computes d_fake = (tanh(noise @ Wg)) @ Wd.

Device sharding (8 cores): shard the N (=H*W) dimension, 2048 columns/core.
Each core computes, for all 1024 samples, the partial dot product
    dpart[b] = sum_{n in shard} tanh((noise @ Wg)[b, n]) * Wd[n]
The host sums the 8 partials, applies softplus and the scalar tail.

Per-core device program (layout: n on PSUM partitions, b on free axis), all
matmuls in fp8e4m3 with perf_mode=DoubleRow (K=256 in one pass, 0.5 cyc/row):
    x[n, b]  = sum_z Wg[z, n] * noiseT[z, b]     (PE, fp8 DoubleRow)
    t[n, b]  = tanh(x[n, b] / 1024)              (ACT, PSUM->SBUF, fp8 out)
    dpart[b] = sum_n Wd[n] * t[n, b]             (PE, fp8 DoubleRow, Wd stationary)
Inputs are pre-scaled on the host (noise*8, Wg*128, Wd*128) to keep fp8
values out of the subnormal range; the ACT scale and a final host divide
undo the scaling.  End-to-end quantization error on the final scalar was
measured at ~6e-4 relative (tolerance 2e-2).

ACT (tanh) is the bottleneck engine: 16384 elem/lane/core at 1.2 GHz.  The
kernel batches tanh into 8 instructions of [128, 2048] (PSUM pair tiles) to
amortize the ~352-cycle per-instruction ACT pipeline fill.
"""

import numpy as np
import ml_dtypes

B, Z, H, W = 1024, 256, 128, 128
N = H * W               # 16384
NCORES = 8
NSH = N // NCORES       # 2048 columns of Wg per core
P = 128
NT = NSH // P           # 16 n-tiles per core
NPAIR = NT // 2         # 8 PSUM pair tiles ([128, 2, 1024] fp32 = 4 banks)
NB = B                  # 1024 samples (free axis)
NQ = 4                  # wg DMA quarters (4 n-tiles each)

# host-side fp8 pre-scales (undone by ACT scale & host divide)
SN = 8.0                # noise scale
SW = 128.0              # Wg scale
SD = 128.0              # Wd scale

# float32 tanh(x) rounds to exactly 1.0 only for x >= ~9.01; stay well below.
WALL_SAFE_BOUND = 8.5

_PROG = None  # cached compiled Bass program


def _build_program():
    import concourse.bass as bass
    import concourse.tile as tile
    from concourse import bacc, mybir

    f32 = mybir.dt.float32
    f8 = mybir.dt.float8e4
    DR = mybir.MatmulPerfMode.DoubleRow
    Tanh = mybir.ActivationFunctionType.Tanh

    nc = bacc.Bacc(
        "TRN2", target_bir_lowering=False, debug=False, num_devices=NCORES
    )
    # Host-relaid inputs (all fp8e4m3, DoubleRow z-interleaved):
    #   noise_t : [128, 2, 1024]      noise_t[k,i,b] = noise[b, 128i+k] * SN
    #   wg_shard: [NQ, 128, 2, 512]   [q,k,i,c]      = Wg[128i+k, 2048*core+512q+c] * SW
    #   wd_shard: [128, 2, 16]        [k,i,p<8]      = Wd[2048*core+256p+128i+k] * SD
    noise_d = nc.declare_dram_parameter("noise_t", [P, 2, NB], f8, isOutput=False)
    wg_d = nc.declare_dram_parameter("wg_shard", [NQ, P, 2, 512], f8, isOutput=False)
    wd_d = nc.declare_dram_parameter("wd_shard", [P, 2, 16], f8, isOutput=False)
    # dpart[1024p + b] = pair-p partial for sample b; host sums over p.
    out_d = nc.declare_dram_parameter("dpart", [1, NPAIR * NB], f32, isOutput=True)

    with tile.TileContext(nc) as tc:
        with (
            tc.tile_pool(name="const", bufs=1) as cpool,
            tc.tile_pool(name="wg", bufs=1) as wgpool,
            tc.tile_pool(name="t", bufs=3) as tpool,
            tc.tile_pool(name="ps", bufs=1, space="PSUM") as pspool,
        ):
            # Two persistent PSUM pair tiles (4 banks each = all 8 banks).
            ps_tiles = [
                pspool.tile([P, 2, NB], f32, name=f"ps{i}", tag=f"ps{i}")
                for i in range(2)
            ]

            # --- PE warm-up: HAM-unthrottle the tensor engine during the DMA
            # wait using matmuls on a memset tile (no DMA dependency).
            warm_sb = cpool.tile([P, 512], f32, tag="warm")
            nc.gpsimd.memset(warm_sb[:], 0.0)
            # Preload the tanh activation table (~2.7us) during the DMA wait
            # instead of right before the first real tanh.
            warm_act = cpool.tile([P, 16], f32, tag="warm_act")
            nc.scalar.activation(warm_act[:], warm_sb[:, 0:16], Tanh)
            for _ in range(8):
                nc.tensor.matmul(
                    ps_tiles[0][0:1, 0:1, 0:256],
                    warm_sb[:, 0:1],
                    warm_sb[:, 0:256],
                    start=True,
                    stop=True,
                    skip_group_check=True,
                )

            wg_q = [
                wgpool.tile([P, 2, 512], f8, name=f"wg{q}", tag=f"wg{q}")
                for q in range(NQ)
            ]
            noise_sb = cpool.tile([P, 2, NB], f8, tag="noise")
            wd_sb = cpool.tile([P, 2, 16], f8, tag="wd")
            out_sb = cpool.tile([1, NPAIR * NB], f32, tag="out")

            # DMA queue plan: the blocks gating the first matmuls go at the
            # head of separate queues.
            nc.scalar.dma_start(out=noise_sb[:, :, 0:512], in_=noise_d[:, :, 0:512])
            nc.gpsimd.dma_start(out=noise_sb[:, :, 512:NB], in_=noise_d[:, :, 512:NB])
            nc.sync.dma_start(out=wg_q[0][:], in_=wg_d[0])
            nc.scalar.dma_start(out=wd_sb[:], in_=wd_d[:])
            nc.sync.dma_start(out=wg_q[1][:], in_=wg_d[1])
            nc.gpsimd.dma_start(out=wg_q[2][:], in_=wg_d[2])
            nc.scalar.dma_start(out=wg_q[3][:], in_=wg_d[3])

            t_tiles = []

            def emit_reduce_and_drain(p):
                ps = ps_tiles[p % 2]
                t = t_tiles[p]
                for h in range(2):
                    # out free 512 fp32 = one PSUM bank (banks 0 then 1)
                    nc.tensor.matmul(
                        ps[0:1, 0:1, 512 * h : 512 * h + 512],
                        wd_sb[:, :, p : p + 1],
                        t[:, :, 512 * h : 512 * h + 512],
                        start=True,
                        stop=True,
                        perf_mode=DR,
                        skip_group_check=True,
                    )
                nc.vector.tensor_copy(
                    out_sb[0:1, NB * p : NB * p + NB],
                    ps[0:1, 0, 0:NB],
                )

            for p in range(NPAIR):
                ps = ps_tiles[p % 2]
                q, pl = divmod(p, 2)
                for tt in range(2):
                    lhsT = wg_q[q][:, :, (2 * pl + tt) * P : (2 * pl + tt + 1) * P]
                    for h in range(2):
                        nc.tensor.matmul(
                            ps[:, tt, 512 * h : 512 * h + 512],
                            lhsT,
                            noise_sb[:, :, 512 * h : 512 * h + 512],
                            start=True,
                            stop=True,
                            perf_mode=DR,
                        )
                if p >= 1:
                    emit_reduce_and_drain(p - 1)
                t = tpool.tile([P, 2, NB], f8, name=f"t{p}", tag="t")
                # ps holds 1024*x; tanh(x) = Tanh(ps / 1024)
                nc.scalar.activation(t[:, :, :], ps[:, :, :], Tanh, scale=1.0 / 1024.0)
                t_tiles.append(t)
            emit_reduce_and_drain(NPAIR - 1)

            nc.sync.dma_start(out=out_d[:], in_=out_sb[0:1, :])

    nc.compile()
    return nc


def _get_program():
    global _PROG
    if _PROG is None:
        _PROG = _build_program()
    return _PROG


def _make_in_maps(noise, Wg, Wd):
    f8 = ml_dtypes.float8_e4m3
    # noise_t[k, i, b] = noise[b, 128i + k] * SN
    noise_t = np.ascontiguousarray(
        (noise.T * SN).astype(f8).reshape(2, P, NB).transpose(1, 0, 2)
    )
    in_maps = []
    for c in range(NCORES):
        wg_c = (Wg[:, c * NSH : (c + 1) * NSH] * SW).astype(f8)  # [Z, NSH]
        # wg_shard[q, k, i, col] = Wg[128i + k, 2048c + 512q + col] * SW
        wg_t = np.ascontiguousarray(
            wg_c.reshape(2, P, NQ, 512).transpose(2, 1, 0, 3)
        )
        # wd_shard[k, i, p] = Wd[2048c + 256p + 128i + k] * SD
        seg = (Wd[c * NSH : (c + 1) * NSH, 0] * SD).astype(f8)
        wd_c = np.zeros((P, 2, 16), f8)
        wd_c[:, :, :NPAIR] = seg.reshape(NPAIR, 2, P).transpose(2, 1, 0)
        in_maps.append({"noise_t": noise_t, "wg_shard": wg_t, "wd_shard": wd_c})
    return in_maps


def _dpart_to_dfake(dpart):
    # dpart [1, 8*1024] f32: dpart[0, 1024p + b] = pair-p partial for sample b
    return np.asarray(dpart, np.float64).reshape(NPAIR, NB).sum(axis=0)


def run_device(noise, Wg, Wd, trace=False):
    """Run the SPMD kernel on 8 cores; return (d_fake[B] float64, results)."""
    from concourse.bass_utils import run_bass_kernel_spmd

    nc = _get_program()
    in_maps = _make_in_maps(noise, Wg, Wd)
    res = run_bass_kernel_spmd(nc, in_maps, list(range(NCORES)), trace=trace)
    d_fake = np.zeros(NB, np.float64)
    for r in res.results:
        d_fake += _dpart_to_dfake(r["dpart"])
    return d_fake / SD, res


def _dilate(v):
    out = v.copy()
    out[:-1, :] |= v[1:, :]
    out[1:, :] |= v[:-1, :]
    out[:, :-1] |= v[:, 1:]
    out[:, 1:] |= v[:, :-1]
    return out


def _host_exact_maze_terms(noise, Wg):
    """Fallback (practically unreachable): exact wall/flood-fill computation."""
    solv = 0.0
    wall_total = 0
    for b0 in range(0, B, 64):
        x = noise[b0 : b0 + 64].astype(np.float32) @ Wg.astype(np.float32)
        fake = np.tanh(x).astype(np.float32)
        for j in range(fake.shape[0]):
            maze = fake[j].reshape(H, W)
            wall = maze == np.float32(1.0)
            nwall = int(wall.sum())
            wall_total += nwall
            pen = 0.0
            if float(wall.mean()) > 0.5:
                pen += 1.0
            if nwall >= 3:
                open_ = ~wall
                visited = np.zeros((H, W), bool)
                visited[1, 1] = True
                while True:
                    nv = visited | (_dilate(visited) & open_)
                    if not (nv & ~visited).any():
                        break
                    visited = nv
                wf = wall.astype(np.float32)
                wa = np.zeros((H, W), np.float32)
                wa[:-1, :] += wf[1:, :]
                wa[1:, :] += wf[:-1, :]
                wa[:, :-1] += wf[:, 1:]
                wa[:, 1:] += wf[:, :-1]
                pen += 0.1 * float((visited & (wa >= 3.0)).sum())
            solv += pen
    solv /= B
    cur = wall_total / float(B * H * W)
    return solv, cur


def kernel(**inputs) -> np.ndarray:
    noise = np.asarray(inputs["noise"], np.float32)
    Wg = np.asarray(inputs["Wg"], np.float32)
    Wd = np.asarray(inputs["Wd"], np.float32)
    p = float(np.asarray(inputs["maml_performance"]).reshape(-1)[0])
    cd = float(np.asarray(inputs["current_difficulty"]).reshape(-1)[0])

    d_fake, _ = run_device(noise, Wg, Wd)

    # g_loss = mean(softplus(-d_fake));  0.0 * sum(d_real) == 0 exactly.
    g_loss = float(np.mean(np.logaddexp(0.0, -d_fake)))

    # Wall existence bound: |x[b,n]| <= max_b||noise_b|| * max_n||Wg[:,n]||.
    rn = float(np.sqrt((noise.astype(np.float64) ** 2).sum(axis=1)).max())
    cn = float(np.sqrt((Wg.astype(np.float64) ** 2).sum(axis=0)).max())
    if rn * cn * 1.0001 < WALL_SAFE_BOUND:
        solv, cur = 0.0, 0.0
    else:  # pragma: no cover - requires |pre-tanh| ~ 28 sigma
        solv, cur = _host_exact_maze_terms(noise, Wg)

    w_s = 0.8 if p < 0.4 else (0.4 if p > 0.6 else 0.6)
    w_d = 0.05 if p < 0.4 else (0.2 if p > 0.6 else 0.1)
    difficulty = (cur - cd) ** 2
    loss = g_loss + w_s * solv + w_d * difficulty
    return np.array(loss, dtype=np.float32)


# revision 15
# speedup vs baseline: 1.2265x; 1.2265x over previous
"""Trainium2 Bass kernel for nn_MAMLAwareGANLoss.

Reference computation (B=1024, Z=256, H=W=128, N=H*W=16384):
    fake   = tanh(noise @ Wg)                      # [B, N]
    d_fake = fake @ Wd                             # [B, 1]
    g_loss = mean(softplus(-d_fake))               # (+ 0.0 * sum(d_real) == 0)
    solvability_loss = mean(per-sample flood-fill penalty of (fake == 1.0) walls)
    cur    = mean(fake == 1.0)
    difficulty_loss  = (cur - current_difficulty)^2
    loss   = g_loss + w_s * solvability_loss + w_d * difficulty_loss

Key structural facts used here:
  * real_mazes enters only through `0.0 * sum(d_real)` == exactly 0.0 -> never loaded.
  * "walls" are cells where float32 tanh(x) rounds to exactly 1.0, which requires
    x >= ~9.01.  We prove on the host (Cauchy-Schwarz over the actual inputs:
    max_b ||noise_b|| * max_n ||Wg[:, n]||) that no |x| can exceed the threshold,
    hence wall count == 0 exactly => solvability_loss == 0.0 and cur == 0.0.
    If the bound ever fails we fall back to an exact host recomputation.
  * Therefore the device only computes d_fake = (tanh(noise @ Wg)) @ Wd.

Device sharding (8 cores): shard the N (=H*W) dimension, 2048 columns/core.
Each core computes, for all 1024 samples, the partial dot product
    dpart[b] = sum_{n in shard} tanh((noise @ Wg)[b, n]) * Wd[n]
The host sums the 8 partials, applies softplus and the scalar tail.

Per-core device program (layout: n on PSUM partitions, b on free axis), all
matmuls in fp8e4m3 with perf_mode=DoubleRow (K=256 in one pass, 0.5 cyc/row):
    x[n, b]  = sum_z Wg[z, n] * noiseT[z, b]     (PE, fp8 DoubleRow)
    t[n, b]  = tanh(x[n, b] / 1024)              (ACT, PSUM->SBUF, fp8 out)
    dpart[b] = sum_n Wd[n] * t[n, b]             (PE, fp8 DoubleRow, Wd stationary)
Inputs are pre-scaled on the host (noise*8, Wg*128, Wd*128) to keep fp8
values out of the subnormal range; the ACT scale and a final host divide
undo the scaling.  End-to-end quantization error on the final scalar was
measured at ~6e-4 relative (tolerance 2e-2).

ACT (tanh) is the bottleneck engine: 16384 elem/lane/core at 1.2 GHz.  The
kernel batches tanh into 8 instructions of [128, 2048] (PSUM pair tiles) to
amortize the ~352-cycle per-instruction ACT pipeline fill.
"""

import numpy as np
import ml_dtypes

B, Z, H, W = 1024, 256, 128, 128
N = H * W               # 16384
NCORES = 8
NSH = N // NCORES       # 2048 columns of Wg per core
P = 128
NT = NSH // P           # 16 n-tiles per core
NPAIR = NT // 2         # 8 PSUM pair tiles ([128, 2, 1024] fp32 = 4 banks)
NB = B                  # 1024 samples (free axis)
NQ = 4                  # wg DMA quarters (4 n-tiles each)
MCHUNK = 512            # moving-b per main matmul (psum bank limit: 512 fp32)

# host-side fp8 pre-scales (undone by ACT scale & host divide)
SN = 8.0                # noise scale
SW = 128.0              # Wg scale
SD = 128.0              # Wd scale

# float32 tanh(x) rounds to exactly 1.0 only for x >= ~9.01; stay well below.
WALL_SAFE_BOUND = 8.5

_PROG = None  # cached compiled Bass program


def _build_program():
    import concourse.bass as bass
    import concourse.tile as tile
    from concourse import bacc, mybir

    f32 = mybir.dt.float32
    f8 = mybir.dt.float8e4
    bf16 = mybir.dt.bfloat16
    DR = mybir.MatmulPerfMode.DoubleRow
    Tanh = mybir.ActivationFunctionType.Tanh

    nc = bacc.Bacc(
        "TRN2", target_bir_lowering=False, debug=False, num_devices=NCORES
    )
    # Host-relaid inputs (all fp8e4m3, DoubleRow z-interleaved):
    #   noise_t : [128, 2, 1024]      noise_t[k,i,b] = noise[b, 128i+k] * SN
    #   wg_shard: [NQ, 128, 2, 512]   [q,k,i,c]      = Wg[128i+k, 2048*core+512q+c] * SW
    #   wd_shard: [128, 2, 16]        [k,i,p<8]      = Wd[2048*core+256p+128i+k] * SD
    noise_d = nc.declare_dram_parameter("noise_t", [P, 2, NB], f8, isOutput=False)
    wg_d = nc.declare_dram_parameter("wg_shard", [NQ, P, 2, 512], f8, isOutput=False)
    wd_d = nc.declare_dram_parameter("wd_shard", [P, 2, 16], f8, isOutput=False)
    # dpart[b]: this core's d_fake partial (times SD); host sums over cores.
    out_d = nc.declare_dram_parameter("dpart", [1, NB], f32, isOutput=True)

    with tile.TileContext(nc) as tc:
        with (
            tc.tile_pool(name="const", bufs=1) as cpool,
            tc.tile_pool(name="wg", bufs=1) as wgpool,
            tc.tile_pool(name="t", bufs=3) as tpool,
            tc.tile_pool(name="ps", bufs=3, space="PSUM") as pspool,
            tc.tile_pool(name="acc", bufs=1, space="PSUM") as apool,
        ):
            # 3 rotating main tiles (2 banks each) + persistent accumulator
            # (2 banks, row 0 used) = all 8 PSUM banks.
            acc = apool.tile([P, NB], f32, tag="acc")

            warm_sb = cpool.tile([P, 256], bf16, tag="warm")
            nc.gpsimd.memset(warm_sb[:], 0.0)
            # Preload the tanh activation table (~2.7us) during the DMA wait
            # instead of right before the first real tanh.
            warm_act = cpool.tile([P, 16], f32, tag="warm_act")
            nc.scalar.activation(warm_act[:], warm_sb[:, 0:16], Tanh)
            # Small PE warm-ups: ramp the PE clock during the DMA wait.
            for _ in range(3):
                nc.tensor.matmul(
                    acc[0:1, 0:256],
                    warm_sb[:, 0:1],
                    warm_sb[:, 0:256],
                    start=True,
                    stop=True,
                    skip_group_check=True,
                )

            wg_q = [
                wgpool.tile([P, 2, 512], f8, name=f"wg{q}", tag=f"wg{q}")
                for q in range(NQ)
            ]
            noise_sb = cpool.tile([P, 2, NB], f8, tag="noise")
            wd_sb = cpool.tile([P, 2, 16], f8, tag="wd")
            out_sb = cpool.tile([1, NB], f32, tag="out")

            # DMA plan: sync + gpsimd queues only (a dma_start on the scalar
            # queue would steal ACT-engine time, and every extra queue adds
            # 16 semaphores to the teardown sweep).
            nc.sync.dma_start(out=wg_q[0][:], in_=wg_d[0])
            nc.gpsimd.dma_start(out=noise_sb[:], in_=noise_d[:])
            nc.sync.dma_start(out=wg_q[1][:], in_=wg_d[1])
            nc.gpsimd.dma_start(out=wd_sb[:], in_=wd_d[:])
            nc.sync.dma_start(out=wg_q[2][:], in_=wg_d[2])
            nc.gpsimd.dma_start(out=wg_q[3][:], in_=wg_d[3])

            t_tiles = []
            act_done = 0  # number of ACT instructions emitted

            def emit_main(i):
                # one n-tile: x[n in tile i, all b] into a rotating psum tile
                q, tl = divmod(i, NQ)
                lhsT = wg_q[q][:, :, tl * P : (tl + 1) * P]
                ps = pspool.tile([P, NB], f32, name=f"mm{i}", tag="ps")
                if MCHUNK == NB:
                    nc.tensor.matmul(
                        ps[:, :], lhsT, noise_sb[:, :, :],
                        start=True, stop=True, perf_mode=DR,
                    )
                else:
                    for h in range(NB // MCHUNK):
                        nc.tensor.matmul(
                            ps[:, h * MCHUNK : (h + 1) * MCHUNK],
                            lhsT,
                            noise_sb[:, :, h * MCHUNK : (h + 1) * MCHUNK],
                            start=True, stop=True, perf_mode=DR,
                        )
                return ps

            def emit_act(i, ps):
                # tanh of n-tile i into slot i%2 of pair tile t[i//2]
                if i % 2 == 0:
                    t_tiles.append(tpool.tile([P, 2, NB], f8, name=f"t{i//2}", tag="t"))
                t = t_tiles[i // 2]
                # ps holds 1024*x; tanh(x) = Tanh(ps / 1024)
                nc.scalar.activation(
                    t[:, i % 2, :], ps[:, :], Tanh, scale=1.0 / 1024.0
                )

            def emit_reduce(p):
                # acc[0, b] += sum_{n in pair p} Wd[n] * t[n, b] (psum-resident
                # accumulation across all pairs; each half stays in one bank)
                t = t_tiles[p]
                for h in range(2):
                    nc.tensor.matmul(
                        acc[0:1, 512 * h : 512 * h + 512],
                        wd_sb[:, :, p : p + 1],
                        t[:, :, 512 * h : 512 * h + 512],
                        start=(p == 0),
                        stop=(p == NPAIR - 1),
                        perf_mode=DR,
                        skip_group_check=True,
                    )

            for i in range(NT):
                ps = emit_main(i)
                # reduce(p) is safe once main(2p+4) has been emitted: that
                # main already waited on ACT(2p+1) via the psum pool rotation,
                # so the reduce's dependency is satisfied with no PE bubble.
                if i >= 4 and i % 2 == 0:
                    emit_reduce(i // 2 - 2)
                emit_act(i, ps)
            emit_reduce(NPAIR - 2)
            emit_reduce(NPAIR - 1)

            nc.vector.tensor_copy(out_sb[0:1, :], acc[0:1, 0:NB])
            nc.sync.dma_start(out=out_d[:], in_=out_sb[0:1, :])

    nc.compile()
    return nc


def _get_program():
    global _PROG
    if _PROG is None:
        _PROG = _build_program()
    return _PROG


def _make_in_maps(noise, Wg, Wd):
    f8 = ml_dtypes.float8_e4m3
    # noise_t[k, i, b] = noise[b, 128i + k] * SN
    noise_t = np.ascontiguousarray(
        (noise.T * SN).astype(f8).reshape(2, P, NB).transpose(1, 0, 2)
    )
    in_maps = []
    for c in range(NCORES):
        wg_c = (Wg[:, c * NSH : (c + 1) * NSH] * SW).astype(f8)  # [Z, NSH]
        # wg_shard[q, k, i, col] = Wg[128i + k, 2048c + 512q + col] * SW
        wg_t = np.ascontiguousarray(
            wg_c.reshape(2, P, NQ, 512).transpose(2, 1, 0, 3)
        )
        # wd_shard[k, i, p] = Wd[2048c + 256p + 128i + k] * SD
        seg = (Wd[c * NSH : (c + 1) * NSH, 0] * SD).astype(f8)
        wd_c = np.zeros((P, 2, 16), f8)
        wd_c[:, :, :NPAIR] = seg.reshape(NPAIR, 2, P).transpose(2, 1, 0)
        in_maps.append({"noise_t": noise_t, "wg_shard": wg_t, "wd_shard": wd_c})
    return in_maps


def _dpart_to_dfake(dpart):
    # dpart [1, 1024] f32: this core's (d_fake * SD) partial
    return np.asarray(dpart, np.float64).reshape(NB)


def run_device(noise, Wg, Wd, trace=False):
    """Run the SPMD kernel on 8 cores; return (d_fake[B] float64, results)."""
    from concourse.bass_utils import run_bass_kernel_spmd

    nc = _get_program()
    in_maps = _make_in_maps(noise, Wg, Wd)
    res = run_bass_kernel_spmd(nc, in_maps, list(range(NCORES)), trace=trace)
    d_fake = np.zeros(NB, np.float64)
    for r in res.results:
        d_fake += _dpart_to_dfake(r["dpart"])
    return d_fake / SD, res


def _dilate(v):
    out = v.copy()
    out[:-1, :] |= v[1:, :]
    out[1:, :] |= v[:-1, :]
    out[:, :-1] |= v[:, 1:]
    out[:, 1:] |= v[:, :-1]
    return out


def _host_exact_maze_terms(noise, Wg):
    """Fallback (practically unreachable): exact wall/flood-fill computation."""
    solv = 0.0
    wall_total = 0
    for b0 in range(0, B, 64):
        x = noise[b0 : b0 + 64].astype(np.float32) @ Wg.astype(np.float32)
        fake = np.tanh(x).astype(np.float32)
        for j in range(fake.shape[0]):
            maze = fake[j].reshape(H, W)
            wall = maze == np.float32(1.0)
            nwall = int(wall.sum())
            wall_total += nwall
            pen = 0.0
            if float(wall.mean()) > 0.5:
                pen += 1.0
            if nwall >= 3:
                open_ = ~wall
                visited = np.zeros((H, W), bool)
                visited[1, 1] = True
                while True:
                    nv = visited | (_dilate(visited) & open_)
                    if not (nv & ~visited).any():
                        break
                    visited = nv
                wf = wall.astype(np.float32)
                wa = np.zeros((H, W), np.float32)
                wa[:-1, :] += wf[1:, :]
                wa[1:, :] += wf[:-1, :]
                wa[:, :-1] += wf[:, 1:]
                wa[:, 1:] += wf[:, :-1]
                pen += 0.1 * float((visited & (wa >= 3.0)).sum())
            solv += pen
    solv /= B
    cur = wall_total / float(B * H * W)
    return solv, cur


def kernel(**inputs) -> np.ndarray:
    noise = np.asarray(inputs["noise"], np.float32)
    Wg = np.asarray(inputs["Wg"], np.float32)
    Wd = np.asarray(inputs["Wd"], np.float32)
    p = float(np.asarray(inputs["maml_performance"]).reshape(-1)[0])
    cd = float(np.asarray(inputs["current_difficulty"]).reshape(-1)[0])

    d_fake, _ = run_device(noise, Wg, Wd)

    # g_loss = mean(softplus(-d_fake));  0.0 * sum(d_real) == 0 exactly.
    g_loss = float(np.mean(np.logaddexp(0.0, -d_fake)))

    # Wall existence bound: |x[b,n]| <= max_b||noise_b|| * max_n||Wg[:,n]||.
    rn = float(np.sqrt((noise.astype(np.float64) ** 2).sum(axis=1)).max())
    cn = float(np.sqrt((Wg.astype(np.float64) ** 2).sum(axis=0)).max())
    if rn * cn * 1.0001 < WALL_SAFE_BOUND:
        solv, cur = 0.0, 0.0
    else:  # pragma: no cover - requires |pre-tanh| ~ 28 sigma
        solv, cur = _host_exact_maze_terms(noise, Wg)

    w_s = 0.8 if p < 0.4 else (0.4 if p > 0.6 else 0.6)
    w_d = 0.05 if p < 0.4 else (0.2 if p > 0.6 else 0.1)
    difficulty = (cur - cd) ** 2
    loss = g_loss + w_s * solv + w_d * difficulty
    return np.array(loss, dtype=np.float32)


# revision 21
# speedup vs baseline: 1.3013x; 1.0610x over previous
"""Trainium2 Bass kernel for nn_MAMLAwareGANLoss.

Reference computation (B=1024, Z=256, H=W=128, N=H*W=16384):
    fake   = tanh(noise @ Wg)                      # [B, N]
    d_fake = fake @ Wd                             # [B, 1]
    g_loss = mean(softplus(-d_fake))               # (+ 0.0 * sum(d_real) == 0)
    solvability_loss = mean(per-sample flood-fill penalty of (fake == 1.0) walls)
    cur    = mean(fake == 1.0)
    difficulty_loss  = (cur - current_difficulty)^2
    loss   = g_loss + w_s * solvability_loss + w_d * difficulty_loss

Key structural facts used here:
  * real_mazes enters only through `0.0 * sum(d_real)` == exactly 0.0 -> never loaded.
  * "walls" are cells where float32 tanh(x) rounds to exactly 1.0, which requires
    x >= ~9.01.  We prove on the host (Cauchy-Schwarz over the actual inputs:
    max_b ||noise_b|| * max_n ||Wg[:, n]||) that no |x| can exceed the threshold,
    hence wall count == 0 exactly => solvability_loss == 0.0 and cur == 0.0.
    If the bound ever fails we fall back to an exact host recomputation.
  * Therefore the device only computes d_fake = (tanh(noise @ Wg)) @ Wd.

Device sharding (8 cores): shard the N (=H*W) dimension, 2048 columns/core.
Each core computes, for all 1024 samples, the partial dot product
    dpart[b] = sum_{n in shard} tanh((noise @ Wg)[b, n]) * Wd[n]
The host sums the 8 partials, applies softplus and the scalar tail.

Per-core device program (layout: n on PSUM partitions, b on free axis), all
matmuls in fp8e4m3 with perf_mode=DoubleRow (K=256 in one pass, 0.5 cyc/row):
    x[n, b]  = sum_z Wg[z, n] * noiseT[z, b]     (PE, fp8 DoubleRow)
    t[n, b]  = tanh(x[n, b] / 1024)              (ACT, PSUM->SBUF, fp8 out)
    dpart[b] = sum_n Wd[n] * t[n, b]             (PE, fp8 DoubleRow, Wd stationary)
Inputs are pre-scaled on the host (noise*8, Wg*128, Wd*128) to keep fp8
values out of the subnormal range; the ACT scale and a final host divide
undo the scaling.  End-to-end quantization error on the final scalar was
measured at ~6e-4 relative (tolerance 2e-2).

ACT (tanh) is the bottleneck engine: 16384 elem/lane/core at 1.2 GHz.  The
kernel batches tanh into 8 instructions of [128, 2048] (PSUM pair tiles) to
amortize the ~352-cycle per-instruction ACT pipeline fill.
"""

import numpy as np
import ml_dtypes

B, Z, H, W = 1024, 256, 128, 128
N = H * W               # 16384
NCORES = 8
NSH = N // NCORES       # 2048 columns of Wg per core
P = 128
NT = NSH // P           # 16 n-tiles per core
NPAIR = NT // 2         # 8 PSUM pair tiles ([128, 2, 1024] fp32 = 4 banks)
NB = B                  # 1024 samples (free axis)
NQ = 8                  # wg DMA chunks (2 n-tiles each)
MCHUNK = 512            # moving-b per main matmul (psum bank limit: 512 fp32)

# host-side fp8 pre-scales (undone by ACT scale & host divide)
SN = 8.0                # noise scale
SW = 128.0              # Wg scale
SD = 128.0              # Wd scale

# float32 tanh(x) rounds to exactly 1.0 only for x >= ~9.01; stay well below.
WALL_SAFE_BOUND = 8.5

_PROG = None  # cached compiled Bass program


def _build_program():
    import concourse.bass as bass
    import concourse.tile as tile
    from concourse import bacc, mybir

    f32 = mybir.dt.float32
    f8 = mybir.dt.float8e4
    bf16 = mybir.dt.bfloat16
    DR = mybir.MatmulPerfMode.DoubleRow
    Tanh = mybir.ActivationFunctionType.Tanh

    nc = bacc.Bacc(
        "TRN2", target_bir_lowering=False, debug=False, num_devices=NCORES
    )
    # Host-relaid inputs (all fp8e4m3, DoubleRow z-interleaved):
    #   noise_t : [128, 2, 2, 512]    [k,h,i,u] = noise[512h+u, 128i+k] * SN
    #   wg_shard: [NQ, 128, 2, 256]   [q,k,i,c] = Wg[128i+k, 2048*core+256q+c] * SW
    #   wd_shard: [128, 2, 16]        [k,i,p<8] = Wd[2048*core+256p+128i+k] * SD
    noise_d = nc.declare_dram_parameter("noise_t", [P, 2, 2, 512], f8, isOutput=False)
    wg_d = nc.declare_dram_parameter("wg_shard", [NQ, P, 2, 256], f8, isOutput=False)
    wd_d = nc.declare_dram_parameter("wd_shard", [P, 2, 16], f8, isOutput=False)
    # dpart[b]: this core's d_fake partial (times SD); host sums over cores.
    out_d = nc.declare_dram_parameter("dpart", [1, NB], f32, isOutput=True)

    with tile.TileContext(nc) as tc:
        with (
            tc.tile_pool(name="const", bufs=1) as cpool,
            tc.tile_pool(name="wg", bufs=1) as wgpool,
            tc.tile_pool(name="t", bufs=3) as tpool,
            tc.tile_pool(name="ps", bufs=3, space="PSUM") as pspool,
            tc.tile_pool(name="acc", bufs=1, space="PSUM") as apool,
        ):
            # 3 rotating main tiles (2 banks each) + persistent accumulator
            # (2 banks, row 0 used) = all 8 PSUM banks.
            acc = apool.tile([P, NB], f32, tag="acc")

            warm_sb = cpool.tile([P, 256], bf16, tag="warm")
            nc.gpsimd.memset(warm_sb[:], 0.0)
            # Preload the tanh activation table (~2.7us) during the DMA wait
            # instead of right before the first real tanh.
            warm_act = cpool.tile([P, 16], f32, tag="warm_act")
            nc.scalar.activation(warm_act[:], warm_sb[:, 0:16], Tanh)
            # PE warm-ups: ramp the PE clock during the ~2us DMA wait so the
            # first real matmuls run near full speed.
            for _ in range(7):
                nc.tensor.matmul(
                    acc[0:1, 0:256],
                    warm_sb[:, 0:1],
                    warm_sb[:, 0:256],
                    start=True,
                    stop=True,
                    skip_group_check=True,
                )

            wg_q = [
                wgpool.tile([P, 2, 256], f8, name=f"wg{q}", tag=f"wg{q}")
                for q in range(NQ)
            ]
            noise_sb = cpool.tile([P, 2, 2, 512], f8, tag="noise")
            wd_sb = cpool.tile([P, 2, 16], f8, tag="wd")
            out_sb = cpool.tile([1, NB], f32, tag="out")

            # DMA plan: sync + gpsimd queues only (a dma_start on the scalar
            # queue would steal ACT-engine time, and every extra queue adds
            # to the teardown semaphore sweep).  Arrival deadlines: wg chunk q
            # (n-tiles 2q, 2q+1) is needed at ~(9 + 2q) us; noise half h
            # gates b-half h of every main matmul.
            nc.sync.dma_start(out=wg_q[0][:], in_=wg_d[0])
            nc.gpsimd.dma_start(out=noise_sb[:, 0], in_=noise_d[:, 0])
            nc.sync.dma_start(out=noise_sb[:, 1], in_=noise_d[:, 1])
            nc.gpsimd.dma_start(out=wg_q[1][:], in_=wg_d[1])
            nc.sync.dma_start(out=wg_q[2][:], in_=wg_d[2])
            nc.gpsimd.dma_start(out=wd_sb[:], in_=wd_d[:])
            nc.gpsimd.dma_start(out=wg_q[3][:], in_=wg_d[3])
            nc.sync.dma_start(out=wg_q[4][:], in_=wg_d[4])
            nc.gpsimd.dma_start(out=wg_q[5][:], in_=wg_d[5])
            nc.sync.dma_start(out=wg_q[6][:], in_=wg_d[6])
            nc.gpsimd.dma_start(out=wg_q[7][:], in_=wg_d[7])

            t_tiles = []
            act_done = 0  # number of ACT instructions emitted

            def emit_main(i):
                # one n-tile: x[n in tile i, all b] into a rotating psum tile
                q, tl = divmod(i, 2)
                lhsT = wg_q[q][:, :, tl * P : (tl + 1) * P]
                ps = pspool.tile([P, NB], f32, name=f"mm{i}", tag="ps")
                for h in range(2):
                    nc.tensor.matmul(
                        ps[:, h * 512 : (h + 1) * 512],
                        lhsT,
                        noise_sb[:, h],
                        start=True, stop=True, perf_mode=DR,
                    )
                return ps

            def emit_act(i, ps):
                # tanh of n-tile i into slot i%2 of pair tile t[i//2]
                if i % 2 == 0:
                    t_tiles.append(tpool.tile([P, 2, NB], f8, name=f"t{i//2}", tag="t"))
                t = t_tiles[i // 2]
                # ps holds 1024*x; tanh(x) = Tanh(ps / 1024).  Tile 0 is split
                # in half so the ACT stream starts as soon as the first
                # noise/wg half-products land, before the full tile is done.
                if i == 0:
                    for h in range(2):
                        nc.scalar.activation(
                            t[:, 0, h * 512 : (h + 1) * 512],
                            ps[:, h * 512 : (h + 1) * 512],
                            Tanh,
                            scale=1.0 / 1024.0,
                        )
                else:
                    nc.scalar.activation(
                        t[:, i % 2, :], ps[:, :], Tanh, scale=1.0 / 1024.0
                    )

            def emit_reduce(p):
                # acc[0, b] += sum_{n in pair p} Wd[n] * t[n, b] (psum-resident
                # accumulation across all pairs; each half stays in one bank)
                t = t_tiles[p]
                for h in range(2):
                    nc.tensor.matmul(
                        acc[0:1, 512 * h : 512 * h + 512],
                        wd_sb[:, :, p : p + 1],
                        t[:, :, 512 * h : 512 * h + 512],
                        start=(p == 0),
                        stop=(p == NPAIR - 1),
                        perf_mode=DR,
                        skip_group_check=True,
                    )

            for i in range(NT):
                ps = emit_main(i)
                # reduce(p) is safe once main(2p+4) has been emitted: that
                # main already waited on ACT(2p+1) via the psum pool rotation,
                # so the reduce's dependency is satisfied with no PE bubble.
                if i >= 4 and i % 2 == 0:
                    emit_reduce(i // 2 - 2)
                emit_act(i, ps)
            emit_reduce(NPAIR - 2)
            emit_reduce(NPAIR - 1)

            # Drain the accumulator in halves on two idle engines.
            nc.scalar.copy(out_sb[0:1, 0:512], acc[0:1, 0:512])
            nc.vector.tensor_copy(out_sb[0:1, 512:NB], acc[0:1, 512:NB])
            nc.sync.dma_start(out=out_d[:], in_=out_sb[0:1, :])

    nc.compile()
    return nc


def _get_program():
    global _PROG
    if _PROG is None:
        _PROG = _build_program()
    return _PROG


def _make_in_maps(noise, Wg, Wd):
    f8 = ml_dtypes.float8_e4m3
    # noise_t[k, h, i, u] = noise[512h + u, 128i + k] * SN
    noise_t = np.ascontiguousarray(
        (noise.T * SN).astype(f8).reshape(2, P, 2, 512).transpose(1, 2, 0, 3)
    )
    in_maps = []
    for c in range(NCORES):
        wg_c = (Wg[:, c * NSH : (c + 1) * NSH] * SW).astype(f8)  # [Z, NSH]
        # wg_shard[q, k, i, col] = Wg[128i + k, 2048c + 256q + col] * SW
        wg_t = np.ascontiguousarray(
            wg_c.reshape(2, P, NQ, 256).transpose(2, 1, 0, 3)
        )
        # wd_shard[k, i, p] = Wd[2048c + 256p + 128i + k] * SD
        seg = (Wd[c * NSH : (c + 1) * NSH, 0] * SD).astype(f8)
        wd_c = np.zeros((P, 2, 16), f8)
        wd_c[:, :, :NPAIR] = seg.reshape(NPAIR, 2, P).transpose(2, 1, 0)
        in_maps.append({"noise_t": noise_t, "wg_shard": wg_t, "wd_shard": wd_c})
    return in_maps


def _dpart_to_dfake(dpart):
    # dpart [1, 1024] f32: this core's (d_fake * SD) partial
    return np.asarray(dpart, np.float64).reshape(NB)


def run_device(noise, Wg, Wd, trace=False):
    """Run the SPMD kernel on 8 cores; return (d_fake[B] float64, results)."""
    from concourse.bass_utils import run_bass_kernel_spmd

    nc = _get_program()
    in_maps = _make_in_maps(noise, Wg, Wd)
    res = run_bass_kernel_spmd(nc, in_maps, list(range(NCORES)), trace=trace)
    d_fake = np.zeros(NB, np.float64)
    for r in res.results:
        d_fake += _dpart_to_dfake(r["dpart"])
    return d_fake / SD, res


def _dilate(v):
    out = v.copy()
    out[:-1, :] |= v[1:, :]
    out[1:, :] |= v[:-1, :]
    out[:, :-1] |= v[:, 1:]
    out[:, 1:] |= v[:, :-1]
    return out


def _host_exact_maze_terms(noise, Wg):
    """Fallback (practically unreachable): exact wall/flood-fill computation."""
    solv = 0.0
    wall_total = 0
    for b0 in range(0, B, 64):
        x = noise[b0 : b0 + 64].astype(np.float32) @ Wg.astype(np.float32)
        fake = np.tanh(x).astype(np.float32)
        for j in range(fake.shape[0]):
            maze = fake[j].reshape(H, W)
            wall = maze == np.float32(1.0)
            nwall = int(wall.sum())
            wall_total += nwall
            pen = 0.0
            if float(wall.mean()) > 0.5:
                pen += 1.0
            if nwall >= 3:
                open_ = ~wall
                visited = np.zeros((H, W), bool)
                visited[1, 1] = True
                while True:
                    nv = visited | (_dilate(visited) & open_)
                    if not (nv & ~visited).any():
                        break
                    visited = nv
                wf = wall.astype(np.float32)
                wa = np.zeros((H, W), np.float32)
                wa[:-1, :] += wf[1:, :]
                wa[1:, :] += wf[:-1, :]
                wa[:, :-1] += wf[:, 1:]
                wa[:, 1:] += wf[:, :-1]
                pen += 0.1 * float((visited & (wa >= 3.0)).sum())
            solv += pen
    solv /= B
    cur = wall_total / float(B * H * W)
    return solv, cur


def kernel(**inputs) -> np.ndarray:
    noise = np.asarray(inputs["noise"], np.float32)
    Wg = np.asarray(inputs["Wg"], np.float32)
    Wd = np.asarray(inputs["Wd"], np.float32)
    p = float(np.asarray(inputs["maml_performance"]).reshape(-1)[0])
    cd = float(np.asarray(inputs["current_difficulty"]).reshape(-1)[0])

    d_fake, _ = run_device(noise, Wg, Wd)

    # g_loss = mean(softplus(-d_fake));  0.0 * sum(d_real) == 0 exactly.
    g_loss = float(np.mean(np.logaddexp(0.0, -d_fake)))

    # Wall existence bound: |x[b,n]| <= max_b||noise_b|| * max_n||Wg[:,n]||.
    rn = float(np.sqrt((noise.astype(np.float64) ** 2).sum(axis=1)).max())
    cn = float(np.sqrt((Wg.astype(np.float64) ** 2).sum(axis=0)).max())
    if rn * cn * 1.0001 < WALL_SAFE_BOUND:
        solv, cur = 0.0, 0.0
    else:  # pragma: no cover - requires |pre-tanh| ~ 28 sigma
        solv, cur = _host_exact_maze_terms(noise, Wg)

    w_s = 0.8 if p < 0.4 else (0.4 if p > 0.6 else 0.6)
    w_d = 0.05 if p < 0.4 else (0.2 if p > 0.6 else 0.1)
    difficulty = (cur - cd) ** 2
    loss = g_loss + w_s * solv + w_d * difficulty
    return np.array(loss, dtype=np.float32)


# revision 26
# speedup vs baseline: 1.3938x; 1.0711x over previous
"""Trainium2 Bass kernel for nn_MAMLAwareGANLoss.

Reference computation (B=1024, Z=256, H=W=128, N=H*W=16384):
    fake   = tanh(noise @ Wg)                      # [B, N]
    d_fake = fake @ Wd                             # [B, 1]
    g_loss = mean(softplus(-d_fake))               # (+ 0.0 * sum(d_real) == 0)
    solvability_loss = mean(per-sample flood-fill penalty of (fake == 1.0) walls)
    cur    = mean(fake == 1.0)
    difficulty_loss  = (cur - current_difficulty)^2
    loss   = g_loss + w_s * solvability_loss + w_d * difficulty_loss

Key structural facts used here:
  * real_mazes enters only through `0.0 * sum(d_real)` == exactly 0.0 -> never loaded.
  * "walls" are cells where float32 tanh(x) rounds to exactly 1.0, which requires
    x >= ~9.01.  We prove on the host (Cauchy-Schwarz over the actual inputs:
    max_b ||noise_b|| * max_n ||Wg[:, n]||) that no |x| can exceed the threshold,
    hence wall count == 0 exactly => solvability_loss == 0.0 and cur == 0.0.
    If the bound ever fails we fall back to an exact host recomputation.
  * Therefore the device only computes d_fake = (tanh(noise @ Wg)) @ Wd.

Device sharding (8 cores): shard the N (=H*W) dimension, 2048 columns/core.
Each core computes, for all 1024 samples, the partial dot product
    dpart[b] = sum_{n in shard} tanh((noise @ Wg)[b, n]) * Wd[n]
The host sums the 8 partials, applies softplus and the scalar tail.

Per-core device program (layout: n on PSUM partitions, b on free axis), all
matmuls in fp8e4m3 with perf_mode=DoubleRow (K=256 in one pass, 0.5 cyc/row):
    x[n, b]  = sum_z Wg[z, n] * noiseT[z, b]     (PE, fp8 DoubleRow)
    t[n, b]  = tanh(x[n, b] / 1024)              (ACT, PSUM->SBUF, fp8 out)
    dpart[b] = sum_n Wd[n] * t[n, b]             (PE, fp8 DoubleRow, Wd stationary)
Inputs are pre-scaled on the host (noise*8, Wg*128, Wd*128) to keep fp8
values out of the subnormal range; the ACT scale and a final host divide
undo the scaling.  End-to-end quantization error on the final scalar was
measured at ~6e-4 relative (tolerance 2e-2).

ACT (tanh) is the bottleneck engine: 16384 elem/lane/core at 1.2 GHz.  The
kernel batches tanh into 8 instructions of [128, 2048] (PSUM pair tiles) to
amortize the ~352-cycle per-instruction ACT pipeline fill.
"""

import numpy as np
import ml_dtypes

B, Z, H, W = 1024, 256, 128, 128
N = H * W               # 16384
NCORES = 8
NSH = N // NCORES       # 2048 columns of Wg per core
P = 128
NT = NSH // P           # 16 n-tiles per core
NPAIR = NT // 2         # 8 PSUM pair tiles ([128, 2, 1024] fp32 = 4 banks)
NB = B                  # 1024 samples (free axis)
NQ = 8                  # wg DMA chunks (2 n-tiles each)
MCHUNK = 512            # moving-b per main matmul (psum bank limit: 512 fp32)

# host-side fp8 pre-scales (undone by ACT scale & host divide)
SN = 8.0                # noise scale
SW = 128.0              # Wg scale
SD = 128.0              # Wd scale

# float32 tanh(x) rounds to exactly 1.0 only for x >= ~9.01; stay well below.
WALL_SAFE_BOUND = 8.5

_PROG = None  # cached compiled Bass program

# Tiles whose tanh runs on the (otherwise idle) vector engine via a two-pass
# custom-DVE clamped quintic, offloading the bottleneck ACT engine.
DVE_TILES = (2, 6, 10, 13)
# tanh(x) ~ u*(A + B u^2 + C u^4), u = clamp(x / LAM, -1, 1); fitted with the
# plateau A+B+C pinned at 0.985 so out-of-range inputs saturate harmlessly.
# Error vs tanh on the actual pre-activation distribution: 1.3e-3 rms.
LAM = 2.1
POLY_A, POLY_B, POLY_C = 2.0834268, -2.5167674, 1.4183406

_DVE_OPS = None


def _register_dve_tanh():
    """Register the two custom-DVE ops (clamp+scale, quintic) at runtime.

    concourse's custom-DVE registry is a module-level list; the per-NEFF
    uop table is generated from it in-process at compile time, so appending
    our ops here is enough for compile, CoreSim, and hardware dispatch.
    """
    global _DVE_OPS
    if _DVE_OPS is not None:
        return _DVE_OPS
    import numpy as np
    from concourse import dve_ops
    from concourse.dve_spec import (
        Spec, Src0, C0, C1, C2, One, Bin, AluOp, maxx, minn, sq, lower,
    )
    from concourse.dve_uop import DveOpSpec
    from concourse.dve_table_gen import dve_ver_for

    def mul(a, b):
        return Bin(AluOp.MULTIPLY, a, b)

    # pass1: u = min(max(x * C0, C1), 1)    [C0 = 1/(1024*LAM), C1 = -1.0]
    spec1 = Spec(
        body=minn(maxx(mul(Src0, C0), C1), One),
        reference=lambda in0, in1, s0, s1, imm2: np.minimum(
            np.maximum(in0 * np.float32(s0), np.float32(s1)), np.float32(1.0)
        ).astype(np.float32),
    )
    # pass2: t = ((C2*s + C1)*s + C0) * u, s = u*u   [C0=A, C1=B, C2=C]
    s_node = sq(Src0)
    spec2 = Spec(
        body=mul(
            Bin(AluOp.ADD, mul(Bin(AluOp.ADD, mul(s_node, C2), C1), s_node), C0),
            Src0,
        ),
        reference=lambda in0, in1, s0, s1, imm2: (
            ((np.float32(imm2) * in0 * in0 + np.float32(s1)) * in0 * in0
             + np.float32(s0)) * in0
        ).astype(np.float32),
    )
    ver = dve_ver_for("TRN2")
    ops = []
    for name, spec in [("TANH_CLAMP_ANT", spec1), ("TANH_POLY_ANT", spec2)]:
        if name in dve_ops._SUB_OPCODE_FOR_NAME:
            ops.append(next(o for o in dve_ops.OPS if o.name == name))
            continue
        row = dve_ops._CUSTOM_DVE_ROW_BASE + len(dve_ops.OPS)
        assert row < 0x20
        uops = lower(spec, ver=ver)
        sha = DveOpSpec(name=name, opcode=row, uops=uops, rd1_en=False).sha(ver)
        op = dve_ops.DveOp(name, spec, subdim=False, uops_sha={ver: sha})
        dve_ops.OPS.append(op)
        dve_ops.CUSTOM_DVE_SPECS[name] = spec
        dve_ops._SUB_OPCODE_FOR_NAME[name] = row
        ops.append(op)
    _DVE_OPS = tuple(ops)
    return _DVE_OPS


def _build_program():
    import concourse.bass as bass
    import concourse.tile as tile
    from concourse import bacc, mybir

    f32 = mybir.dt.float32
    f8 = mybir.dt.float8e4
    f16 = mybir.dt.float16
    bf16 = mybir.dt.bfloat16
    DR = mybir.MatmulPerfMode.DoubleRow
    Tanh = mybir.ActivationFunctionType.Tanh
    OP_CLAMP, OP_POLY = _register_dve_tanh()

    nc = bacc.Bacc(
        "TRN2", target_bir_lowering=False, debug=False, num_devices=NCORES
    )
    # Host-relaid inputs (all fp8e4m3, DoubleRow z-interleaved):
    #   noise_t : [128, 2, 2, 512]    [k,h,i,u] = noise[512h+u, 128i+k] * SN
    #   wg_shard: [NQ, 128, 2, 256]   [q,k,i,c] = Wg[128i+k, 2048*core+256q+c] * SW
    #   wd_shard: [128, 2, 16]        [k,i,p<8] = Wd[2048*core+256p+128i+k] * SD
    noise_d = nc.declare_dram_parameter("noise_t", [P, 2, 2, 512], f8, isOutput=False)
    wg_d = nc.declare_dram_parameter("wg_shard", [NQ, P, 2, 256], f8, isOutput=False)
    wd_d = nc.declare_dram_parameter("wd_shard", [P, 2, 16], f8, isOutput=False)
    # dpart[b]: this core's d_fake partial (times SD); host sums over cores.
    out_d = nc.declare_dram_parameter("dpart", [1, NB], f32, isOutput=True)

    with tile.TileContext(nc) as tc:
        with (
            tc.tile_pool(name="const", bufs=1) as cpool,
            tc.tile_pool(name="wg", bufs=1) as wgpool,
            tc.tile_pool(name="t", bufs=3) as tpool,
            tc.tile_pool(name="u", bufs=2) as upool,
            tc.tile_pool(name="ps", bufs=3, space="PSUM") as pspool,
            tc.tile_pool(name="acc", bufs=1, space="PSUM") as apool,
        ):
            # 3 rotating main tiles (2 banks each) + persistent accumulator
            # (2 banks, row 0 used) = all 8 PSUM banks.
            acc = apool.tile([P, NB], f32, tag="acc")

            warm_sb = cpool.tile([P, 256], bf16, tag="warm")
            nc.gpsimd.memset(warm_sb[:], 0.0)
            # Preload the tanh activation table (~2.7us) during the DMA wait
            # instead of right before the first real tanh.
            warm_act = cpool.tile([P, 16], f32, tag="warm_act")
            nc.scalar.activation(warm_act[:], warm_sb[:, 0:16], Tanh)
            # PE warm-ups: ramp the PE clock during the ~2us DMA wait so the
            # first real matmuls run near full speed.
            for _ in range(7):
                nc.tensor.matmul(
                    acc[0:1, 0:256],
                    warm_sb[:, 0:1],
                    warm_sb[:, 0:256],
                    start=True,
                    stop=True,
                    skip_group_check=True,
                )

            wg_q = [
                wgpool.tile([P, 2, 256], f8, name=f"wg{q}", tag=f"wg{q}")
                for q in range(NQ)
            ]
            noise_sb = cpool.tile([P, 2, 2, 512], f8, tag="noise")
            wd_sb = cpool.tile([P, 2, 16], f8, tag="wd")
            out_sb = cpool.tile([1, NB], f32, tag="out")

            # DMA plan: sync + gpsimd queues only (a dma_start on the scalar
            # queue would steal ACT-engine time, and every extra queue adds
            # to the teardown semaphore sweep).  The sync queue starts issuing
            # ~0.7us before gpsimd (gpsimd runs the memsets first), so the
            # larger noise half goes at the sync head and wg chunk 0 on
            # gpsimd; everything else streams behind well ahead of its use.
            nc.sync.dma_start(out=noise_sb[:, 0], in_=noise_d[:, 0])
            nc.gpsimd.dma_start(out=wg_q[0][:], in_=wg_d[0])
            nc.sync.dma_start(out=noise_sb[:, 1], in_=noise_d[:, 1])
            nc.gpsimd.dma_start(out=wg_q[1][:], in_=wg_d[1])
            nc.gpsimd.dma_start(out=wd_sb[:], in_=wd_d[:])
            nc.sync.dma_start(out=wg_q[2][:], in_=wg_d[2])
            nc.gpsimd.dma_start(out=wg_q[3][:], in_=wg_d[3])
            nc.sync.dma_start(out=wg_q[4][:], in_=wg_d[4])
            nc.gpsimd.dma_start(out=wg_q[5][:], in_=wg_d[5])
            nc.sync.dma_start(out=wg_q[6][:], in_=wg_d[6])
            nc.gpsimd.dma_start(out=wg_q[7][:], in_=wg_d[7])

            t_tiles = []
            act_done = 0  # number of ACT instructions emitted

            def get_t(i):
                if i % 2 == 0:
                    t_tiles.append(tpool.tile([P, 2, NB], f8, name=f"t{i//2}", tag="t"))
                return t_tiles[i // 2]

            def emit_main(i, interleave_act=False):
                # one n-tile: x[n in tile i, all b] into a rotating psum tile
                q, tl = divmod(i, 2)
                lhsT = wg_q[q][:, :, tl * P : (tl + 1) * P]
                ps = pspool.tile([P, NB], f32, name=f"mm{i}", tag="ps")
                for h in range(2):
                    nc.tensor.matmul(
                        ps[:, h * 512 : (h + 1) * 512],
                        lhsT,
                        noise_sb[:, h],
                        start=True, stop=True, perf_mode=DR,
                    )
                    if interleave_act:
                        # tile 0: emit the half-tile tanh right after its
                        # matmul so ACT starts before the other noise half
                        # has even arrived.
                        nc.scalar.activation(
                            get_t(i)[:, 0, h * 512 : (h + 1) * 512],
                            ps[:, h * 512 : (h + 1) * 512],
                            Tanh,
                            scale=1.0 / 1024.0,
                        )
                return ps

            def emit_act(i, ps):
                # tanh of n-tile i into slot i%2 of pair tile t[i//2]
                # (ps holds 1024*x; tanh(x) = Tanh(ps / 1024))
                nc.scalar.activation(
                    get_t(i)[:, i % 2, :], ps[:, :], Tanh, scale=1.0 / 1024.0
                )

            def emit_dve_tanh(i, ps):
                # two-pass clamped-quintic tanh on the vector engine
                t = get_t(i)
                u = upool.tile([P, NB], f16, name=f"u{i}", tag="u")
                nc.vector._custom_dve(
                    OP_CLAMP, out=u[:, :], in0=ps[:, :],
                    s0=1.0 / (1024.0 * LAM), s1=-1.0,
                )
                nc.vector._custom_dve(
                    OP_POLY, out=t[:, i % 2, :], in0=u[:, :],
                    s0=POLY_A, s1=POLY_B, imm2=POLY_C,
                )

            def emit_reduce(p):
                # acc[0, b] += sum_{n in pair p} Wd[n] * t[n, b] (psum-resident
                # accumulation across all pairs; each half stays in one bank)
                t = t_tiles[p]
                for h in range(2):
                    nc.tensor.matmul(
                        acc[0:1, 512 * h : 512 * h + 512],
                        wd_sb[:, :, p : p + 1],
                        t[:, :, 512 * h : 512 * h + 512],
                        start=(p == 0),
                        stop=(p == NPAIR - 1),
                        perf_mode=DR,
                        skip_group_check=True,
                    )

            for i in range(NT):
                ps = emit_main(i, interleave_act=(i == 0))
                # reduce(p) is emitted once main(2p+6) is out: that main's
                # psum-pool wait already implies the pair's tanh consumers
                # are done, so the reduce adds no PE bubble.
                if i >= 6 and i % 2 == 0:
                    emit_reduce((i - 6) // 2)
                if i == 0:
                    pass  # tanh already interleaved with the matmuls
                elif i in DVE_TILES:
                    emit_dve_tanh(i, ps)
                else:
                    emit_act(i, ps)
            for p in range(NPAIR - 3, NPAIR):
                emit_reduce(p)

            # Drain the accumulator in halves on two idle engines.
            nc.scalar.copy(out_sb[0:1, 0:512], acc[0:1, 0:512])
            nc.vector.tensor_copy(out_sb[0:1, 512:NB], acc[0:1, 512:NB])
            nc.sync.dma_start(out=out_d[:], in_=out_sb[0:1, :])

    nc.compile()
    return nc


def _get_program():
    global _PROG
    if _PROG is None:
        _PROG = _build_program()
    return _PROG


def _make_in_maps(noise, Wg, Wd):
    f8 = ml_dtypes.float8_e4m3
    # noise_t[k, h, i, u] = noise[512h + u, 128i + k] * SN
    noise_t = np.ascontiguousarray(
        (noise.T * SN).astype(f8).reshape(2, P, 2, 512).transpose(1, 2, 0, 3)
    )
    in_maps = []
    for c in range(NCORES):
        wg_c = (Wg[:, c * NSH : (c + 1) * NSH] * SW).astype(f8)  # [Z, NSH]
        # wg_shard[q, k, i, col] = Wg[128i + k, 2048c + 256q + col] * SW
        wg_t = np.ascontiguousarray(
            wg_c.reshape(2, P, NQ, 256).transpose(2, 1, 0, 3)
        )
        # wd_shard[k, i, p] = Wd[2048c + 256p + 128i + k] * SD
        seg = (Wd[c * NSH : (c + 1) * NSH, 0] * SD).astype(f8)
        wd_c = np.zeros((P, 2, 16), f8)
        wd_c[:, :, :NPAIR] = seg.reshape(NPAIR, 2, P).transpose(2, 1, 0)
        in_maps.append({"noise_t": noise_t, "wg_shard": wg_t, "wd_shard": wd_c})
    return in_maps


def _dpart_to_dfake(dpart):
    # dpart [1, 1024] f32: this core's (d_fake * SD) partial
    return np.asarray(dpart, np.float64).reshape(NB)


def run_device(noise, Wg, Wd, trace=False):
    """Run the SPMD kernel on 8 cores; return (d_fake[B] float64, results)."""
    from concourse.bass_utils import run_bass_kernel_spmd

    nc = _get_program()
    in_maps = _make_in_maps(noise, Wg, Wd)
    res = run_bass_kernel_spmd(nc, in_maps, list(range(NCORES)), trace=trace)
    d_fake = np.zeros(NB, np.float64)
    for r in res.results:
        d_fake += _dpart_to_dfake(r["dpart"])
    return d_fake / SD, res


def _dilate(v):
    out = v.copy()
    out[:-1, :] |= v[1:, :]
    out[1:, :] |= v[:-1, :]
    out[:, :-1] |= v[:, 1:]
    out[:, 1:] |= v[:, :-1]
    return out


def _host_exact_maze_terms(noise, Wg):
    """Fallback (practically unreachable): exact wall/flood-fill computation."""
    solv = 0.0
    wall_total = 0
    for b0 in range(0, B, 64):
        x = noise[b0 : b0 + 64].astype(np.float32) @ Wg.astype(np.float32)
        fake = np.tanh(x).astype(np.float32)
        for j in range(fake.shape[0]):
            maze = fake[j].reshape(H, W)
            wall = maze == np.float32(1.0)
            nwall = int(wall.sum())
            wall_total += nwall
            pen = 0.0
            if float(wall.mean()) > 0.5:
                pen += 1.0
            if nwall >= 3:
                open_ = ~wall
                visited = np.zeros((H, W), bool)
                visited[1, 1] = True
                while True:
                    nv = visited | (_dilate(visited) & open_)
                    if not (nv & ~visited).any():
                        break
                    visited = nv
                wf = wall.astype(np.float32)
                wa = np.zeros((H, W), np.float32)
                wa[:-1, :] += wf[1:, :]
                wa[1:, :] += wf[:-1, :]
                wa[:, :-1] += wf[:, 1:]
                wa[:, 1:] += wf[:, :-1]
                pen += 0.1 * float((visited & (wa >= 3.0)).sum())
            solv += pen
    solv /= B
    cur = wall_total / float(B * H * W)
    return solv, cur


def kernel(**inputs) -> np.ndarray:
    noise = np.asarray(inputs["noise"], np.float32)
    Wg = np.asarray(inputs["Wg"], np.float32)
    Wd = np.asarray(inputs["Wd"], np.float32)
    p = float(np.asarray(inputs["maml_performance"]).reshape(-1)[0])
    cd = float(np.asarray(inputs["current_difficulty"]).reshape(-1)[0])

    d_fake, _ = run_device(noise, Wg, Wd)

    # g_loss = mean(softplus(-d_fake));  0.0 * sum(d_real) == 0 exactly.
    g_loss = float(np.mean(np.logaddexp(0.0, -d_fake)))

    # Wall existence bound: |x[b,n]| <= max_b||noise_b|| * max_n||Wg[:,n]||.
    rn = float(np.sqrt((noise.astype(np.float64) ** 2).sum(axis=1)).max())
    cn = float(np.sqrt((Wg.astype(np.float64) ** 2).sum(axis=0)).max())
    if rn * cn * 1.0001 < WALL_SAFE_BOUND:
        solv, cur = 0.0, 0.0
    else:  # pragma: no cover - requires |pre-tanh| ~ 28 sigma
        solv, cur = _host_exact_maze_terms(noise, Wg)

    w_s = 0.8 if p < 0.4 else (0.4 if p > 0.6 else 0.6)
    w_d = 0.05 if p < 0.4 else (0.2 if p > 0.6 else 0.1)
    difficulty = (cur - cd) ** 2
    loss = g_loss + w_s * solv + w_d * difficulty
    return np.array(loss, dtype=np.float32)


# revision 31
# speedup vs baseline: 1.4402x; 1.0333x over previous
"""Trainium2 Bass kernel for nn_MAMLAwareGANLoss.

Reference computation (B=1024, Z=256, H=W=128, N=H*W=16384):
    fake   = tanh(noise @ Wg)                      # [B, N]
    d_fake = fake @ Wd                             # [B, 1]
    g_loss = mean(softplus(-d_fake))               # (+ 0.0 * sum(d_real) == 0)
    solvability_loss = mean(per-sample flood-fill penalty of (fake == 1.0) walls)
    cur    = mean(fake == 1.0)
    difficulty_loss  = (cur - current_difficulty)^2
    loss   = g_loss + w_s * solvability_loss + w_d * difficulty_loss

Key structural facts used here:
  * real_mazes enters only through `0.0 * sum(d_real)` == exactly 0.0 -> never loaded.
  * "walls" are cells where float32 tanh(x) rounds to exactly 1.0, which requires
    x >= ~9.01.  We prove on the host (Cauchy-Schwarz over the actual inputs:
    max_b ||noise_b|| * max_n ||Wg[:, n]||) that no |x| can exceed the threshold,
    hence wall count == 0 exactly => solvability_loss == 0.0 and cur == 0.0.
    If the bound ever fails we fall back to an exact host recomputation.
  * Therefore the device only computes d_fake = (tanh(noise @ Wg)) @ Wd.

Device sharding (8 cores): shard the N (=H*W) dimension, 2048 columns/core.
Each core computes, for all 1024 samples, the partial dot product
    dpart[b] = sum_{n in shard} tanh((noise @ Wg)[b, n]) * Wd[n]
The host sums the 8 partials, applies softplus and the scalar tail.

Per-core device program (layout: n on PSUM partitions, b on free axis), all
matmuls in fp8e4m3 with perf_mode=DoubleRow (K=256 in one pass, 0.5 cyc/row):
    x[n, b]  = sum_z Wg[z, n] * noiseT[z, b]     (PE, fp8 DoubleRow)
    t[n, b]  = tanh(x[n, b] / 1024)              (ACT, PSUM->SBUF, fp8 out)
    dpart[b] = sum_n Wd[n] * t[n, b]             (PE, fp8 DoubleRow, Wd stationary)
Inputs are pre-scaled on the host (noise*8, Wg*128, Wd*128) to keep fp8
values out of the subnormal range; the ACT scale and a final host divide
undo the scaling.  End-to-end quantization error on the final scalar was
measured at ~6e-4 relative (tolerance 2e-2).

ACT (tanh) is the bottleneck engine: 16384 elem/lane/core at 1.2 GHz.  The
kernel batches tanh into 8 instructions of [128, 2048] (PSUM pair tiles) to
amortize the ~352-cycle per-instruction ACT pipeline fill.
"""

import numpy as np
import ml_dtypes

B, Z, H, W = 1024, 256, 128, 128
N = H * W               # 16384
NCORES = 8
NSH = N // NCORES       # 2048 columns of Wg per core
P = 128
NT = NSH // P           # 16 n-tiles per core
NPAIR = NT // 2         # 8 PSUM pair tiles ([128, 2, 1024] fp32 = 4 banks)
NB = B                  # 1024 samples (free axis)
NQ = 8                  # wg DMA chunks (2 n-tiles each)
MCHUNK = 512            # moving-b per main matmul (psum bank limit: 512 fp32)

# host-side fp8 pre-scales (undone by ACT scale & host divide)
SN = 8.0                # noise scale
SW = 128.0              # Wg scale
SD = 128.0              # Wd scale

# float32 tanh(x) rounds to exactly 1.0 only for x >= ~9.01; stay well below.
WALL_SAFE_BOUND = 8.5

_PROG = None  # cached compiled Bass program

# Tiles whose tanh runs on the (otherwise idle) vector engine via a two-pass
# custom-DVE clamped quintic, offloading the bottleneck ACT engine.
DVE_TILES = (2, 6, 10, 13)
# tanh(x) ~ u*(A + B u^2 + C u^4), u = clamp(x / LAM, -1, 1); fitted with the
# plateau A+B+C pinned at 0.985 so out-of-range inputs saturate harmlessly.
# Error vs tanh on the actual pre-activation distribution: 1.3e-3 rms.
LAM = 2.1
POLY_A, POLY_B, POLY_C = 2.0834268, -2.5167674, 1.4183406

_DVE_OPS = None


def _register_dve_tanh():
    """Register the two custom-DVE ops (clamp+scale, quintic) at runtime.

    concourse's custom-DVE registry is a module-level list; the per-NEFF
    uop table is generated from it in-process at compile time, so appending
    our ops here is enough for compile, CoreSim, and hardware dispatch.
    """
    global _DVE_OPS
    if _DVE_OPS is not None:
        return _DVE_OPS
    import numpy as np
    from concourse import dve_ops
    from concourse.dve_spec import (
        Spec, Src0, C0, C1, C2, One, Bin, AluOp, maxx, minn, sq, lower,
    )
    from concourse.dve_uop import DveOpSpec
    from concourse.dve_table_gen import dve_ver_for

    def mul(a, b):
        return Bin(AluOp.MULTIPLY, a, b)

    # pass1: u = min(max(x * C0, C1), 1)    [C0 = 1/(1024*LAM), C1 = -1.0]
    spec1 = Spec(
        body=minn(maxx(mul(Src0, C0), C1), One),
        reference=lambda in0, in1, s0, s1, imm2: np.minimum(
            np.maximum(in0 * np.float32(s0), np.float32(s1)), np.float32(1.0)
        ).astype(np.float32),
    )
    # pass2: t = ((C2*s + C1)*s + C0) * u, s = u*u   [C0=A, C1=B, C2=C]
    s_node = sq(Src0)
    spec2 = Spec(
        body=mul(
            Bin(AluOp.ADD, mul(Bin(AluOp.ADD, mul(s_node, C2), C1), s_node), C0),
            Src0,
        ),
        reference=lambda in0, in1, s0, s1, imm2: (
            ((np.float32(imm2) * in0 * in0 + np.float32(s1)) * in0 * in0
             + np.float32(s0)) * in0
        ).astype(np.float32),
    )
    ver = dve_ver_for("TRN2")
    ops = []
    for name, spec in [("TANH_CLAMP_ANT", spec1), ("TANH_POLY_ANT", spec2)]:
        if name in dve_ops._SUB_OPCODE_FOR_NAME:
            ops.append(next(o for o in dve_ops.OPS if o.name == name))
            continue
        row = dve_ops._CUSTOM_DVE_ROW_BASE + len(dve_ops.OPS)
        assert row < 0x20
        uops = lower(spec, ver=ver)
        sha = DveOpSpec(name=name, opcode=row, uops=uops, rd1_en=False).sha(ver)
        op = dve_ops.DveOp(name, spec, subdim=False, uops_sha={ver: sha})
        dve_ops.OPS.append(op)
        dve_ops.CUSTOM_DVE_SPECS[name] = spec
        dve_ops._SUB_OPCODE_FOR_NAME[name] = row
        ops.append(op)
    _DVE_OPS = tuple(ops)
    return _DVE_OPS


def _build_program():
    import concourse.bass as bass
    import concourse.tile as tile
    from concourse import bacc, mybir

    f32 = mybir.dt.float32
    f8 = mybir.dt.float8e4
    f16 = mybir.dt.float16
    bf16 = mybir.dt.bfloat16
    DR = mybir.MatmulPerfMode.DoubleRow
    Tanh = mybir.ActivationFunctionType.Tanh
    OP_CLAMP, OP_POLY = _register_dve_tanh()

    nc = bacc.Bacc(
        "TRN2", target_bir_lowering=False, debug=False, num_devices=NCORES
    )
    # Host-relaid inputs (all fp8e4m3, DoubleRow z-interleaved):
    #   noise_t : [128, 2, 2, 512]    [k,h,i,u] = noise[512h+u, 128i+k] * SN
    #   wg_shard: [NQ, 128, 2, 256]   [q,k,i,c] = Wg[128i+k, 2048*core+256q+c] * SW
    #   wd_shard: [128, 2, 16]        [k,i,p<8] = Wd[2048*core+256p+128i+k] * SD
    noise_d = nc.declare_dram_parameter("noise_t", [P, 2, 2, 512], f8, isOutput=False)
    wg_d = nc.declare_dram_parameter("wg_shard", [NQ, P, 2, 256], f8, isOutput=False)
    wd_d = nc.declare_dram_parameter("wd_shard", [P, 2, 16], f8, isOutput=False)
    # dpart[b]: this core's d_fake partial (times SD); host sums over cores.
    out_d = nc.declare_dram_parameter("dpart", [1, NB], f32, isOutput=True)

    with tile.TileContext(nc) as tc:
        with (
            tc.tile_pool(name="const", bufs=1) as cpool,
            tc.tile_pool(name="wg", bufs=1) as wgpool,
            tc.tile_pool(name="t", bufs=3) as tpool,
            tc.tile_pool(name="u", bufs=2) as upool,
            tc.tile_pool(name="ps", bufs=3, space="PSUM") as pspool,
            tc.tile_pool(name="acc", bufs=1, space="PSUM") as apool,
        ):
            # 3 rotating main tiles (2 banks each) + persistent accumulator
            # (2 banks, row 0 used) = all 8 PSUM banks.
            acc = apool.tile([P, NB], f32, tag="acc")

            warm_sb = cpool.tile([P, 256], bf16, tag="warm")
            nc.gpsimd.memset(warm_sb[:], 0.0)
            # Preload the tanh activation table (~2.7us) during the DMA wait
            # instead of right before the first real tanh.
            warm_act = cpool.tile([P, 16], f32, tag="warm_act")
            nc.scalar.activation(warm_act[:], warm_sb[:, 0:16], Tanh)
            # PE warm-ups: keep the PE busy through the whole ~2.2us DMA wait
            # so its p-state is fully ramped when the first real matmul lands
            # (an idle PE decays back to the 1.2 GHz mid state).
            for _ in range(10):
                nc.tensor.matmul(
                    acc[0:1, 0:256],
                    warm_sb[:, 0:1],
                    warm_sb[:, 0:256],
                    start=True,
                    stop=True,
                    skip_group_check=True,
                )

            wg_q = [
                wgpool.tile([P, 2, 256], f8, name=f"wg{q}", tag=f"wg{q}")
                for q in range(NQ)
            ]
            noise_sb = cpool.tile([P, 2, 2, 512], f8, tag="noise")
            wd_sb = cpool.tile([P, 2, 16], f8, tag="wd")
            out_sb = cpool.tile([1, NB], f32, tag="out")

            # DMA plan.  The head transfers (noise + wg chunk 0) gate the
            # whole pipeline, so they are spread across all three DGE queues:
            # sync carries noise b[0:512] + b[512:768], scalar (after the ACT
            # table preload, which must come first on that engine) carries
            # noise b[768:1024], gpsimd carries the wg chunks.  The teardown
            # semaphore sweep is a fixed range, so extra queues cost nothing.
            nc.sync.dma_start(out=noise_sb[:, 0], in_=noise_d[:, 0])
            nc.gpsimd.dma_start(out=wg_q[0][:], in_=wg_d[0])
            nc.sync.dma_start(
                out=noise_sb[:, 1, 0:1, :], in_=noise_d[:, 1, 0:1, :]
            )
            nc.scalar.dma_start(
                out=noise_sb[:, 1, 1:2, :], in_=noise_d[:, 1, 1:2, :]
            )
            nc.gpsimd.dma_start(out=wg_q[1][:], in_=wg_d[1])
            nc.gpsimd.dma_start(out=wd_sb[:], in_=wd_d[:])
            nc.gpsimd.dma_start(out=wg_q[2][:], in_=wg_d[2])
            nc.gpsimd.dma_start(out=wg_q[3][:], in_=wg_d[3])
            nc.sync.dma_start(out=wg_q[4][:], in_=wg_d[4])
            nc.gpsimd.dma_start(out=wg_q[5][:], in_=wg_d[5])
            nc.sync.dma_start(out=wg_q[6][:], in_=wg_d[6])
            nc.gpsimd.dma_start(out=wg_q[7][:], in_=wg_d[7])

            t_tiles = []
            act_done = 0  # number of ACT instructions emitted

            def get_t(i):
                if i % 2 == 0:
                    t_tiles.append(tpool.tile([P, 2, NB], f8, name=f"t{i//2}", tag="t"))
                return t_tiles[i // 2]

            def emit_main(i, interleave_act=False):
                # one n-tile: x[n in tile i, all b] into a rotating psum tile
                q, tl = divmod(i, 2)
                lhsT = wg_q[q][:, :, tl * P : (tl + 1) * P]
                ps = pspool.tile([P, NB], f32, name=f"mm{i}", tag="ps")
                for h in range(2):
                    nc.tensor.matmul(
                        ps[:, h * 512 : (h + 1) * 512],
                        lhsT,
                        noise_sb[:, h],
                        start=True, stop=True, perf_mode=DR,
                    )
                    if interleave_act:
                        # first tiles: emit the half-tile tanh right after
                        # its matmul so ACT starts before the second noise
                        # half has even arrived.
                        nc.scalar.activation(
                            get_t(i)[:, i % 2, h * 512 : (h + 1) * 512],
                            ps[:, h * 512 : (h + 1) * 512],
                            Tanh,
                            scale=1.0 / 1024.0,
                        )
                return ps

            def emit_act(i, ps):
                # tanh of n-tile i into slot i%2 of pair tile t[i//2]
                # (ps holds 1024*x; tanh(x) = Tanh(ps / 1024))
                nc.scalar.activation(
                    get_t(i)[:, i % 2, :], ps[:, :], Tanh, scale=1.0 / 1024.0
                )

            def emit_dve_tanh(i, ps):
                # two-pass clamped-quintic tanh on the vector engine
                t = get_t(i)
                u = upool.tile([P, NB], f16, name=f"u{i}", tag="u")
                nc.vector._custom_dve(
                    OP_CLAMP, out=u[:, :], in0=ps[:, :],
                    s0=1.0 / (1024.0 * LAM), s1=-1.0,
                )
                nc.vector._custom_dve(
                    OP_POLY, out=t[:, i % 2, :], in0=u[:, :],
                    s0=POLY_A, s1=POLY_B, imm2=POLY_C,
                )

            def emit_reduce(p):
                # acc[0, b] += sum_{n in pair p} Wd[n] * t[n, b] (psum-resident
                # accumulation across all pairs; each half stays in one bank)
                t = t_tiles[p]
                for h in range(2):
                    nc.tensor.matmul(
                        acc[0:1, 512 * h : 512 * h + 512],
                        wd_sb[:, :, p : p + 1],
                        t[:, :, 512 * h : 512 * h + 512],
                        start=(p == 0),
                        stop=(p == NPAIR - 1),
                        perf_mode=DR,
                        skip_group_check=True,
                    )

            for i in range(NT):
                ps = emit_main(i, interleave_act=(i <= 1))
                # reduce(p) is emitted once main(2p+6) is out: that main's
                # psum-pool wait already implies the pair's tanh consumers
                # are done, so the reduce adds no PE bubble.
                if i >= 6 and i % 2 == 0:
                    emit_reduce((i - 6) // 2)
                if i <= 1:
                    pass  # tanh already interleaved with the matmuls
                elif i in DVE_TILES:
                    emit_dve_tanh(i, ps)
                else:
                    emit_act(i, ps)
            for p in range(NPAIR - 3, NPAIR):
                emit_reduce(p)

            # Drain the accumulator in halves on two idle engines.
            nc.scalar.copy(out_sb[0:1, 0:512], acc[0:1, 0:512])
            nc.vector.tensor_copy(out_sb[0:1, 512:NB], acc[0:1, 512:NB])
            nc.sync.dma_start(out=out_d[:], in_=out_sb[0:1, :])

    nc.compile()
    return nc


def _get_program():
    global _PROG
    if _PROG is None:
        _PROG = _build_program()
    return _PROG


def _make_in_maps(noise, Wg, Wd):
    f8 = ml_dtypes.float8_e4m3
    # noise_t[k, h, i, u] = noise[512h + u, 128i + k] * SN
    noise_t = np.ascontiguousarray(
        (noise.T * SN).astype(f8).reshape(2, P, 2, 512).transpose(1, 2, 0, 3)
    )
    in_maps = []
    for c in range(NCORES):
        wg_c = (Wg[:, c * NSH : (c + 1) * NSH] * SW).astype(f8)  # [Z, NSH]
        # wg_shard[q, k, i, col] = Wg[128i + k, 2048c + 256q + col] * SW
        wg_t = np.ascontiguousarray(
            wg_c.reshape(2, P, NQ, 256).transpose(2, 1, 0, 3)
        )
        # wd_shard[k, i, p] = Wd[2048c + 256p + 128i + k] * SD
        seg = (Wd[c * NSH : (c + 1) * NSH, 0] * SD).astype(f8)
        wd_c = np.zeros((P, 2, 16), f8)
        wd_c[:, :, :NPAIR] = seg.reshape(NPAIR, 2, P).transpose(2, 1, 0)
        in_maps.append({"noise_t": noise_t, "wg_shard": wg_t, "wd_shard": wd_c})
    return in_maps


def _dpart_to_dfake(dpart):
    # dpart [1, 1024] f32: this core's (d_fake * SD) partial
    return np.asarray(dpart, np.float64).reshape(NB)


def run_device(noise, Wg, Wd, trace=False):
    """Run the SPMD kernel on 8 cores; return (d_fake[B] float64, results)."""
    from concourse.bass_utils import run_bass_kernel_spmd

    nc = _get_program()
    in_maps = _make_in_maps(noise, Wg, Wd)
    res = run_bass_kernel_spmd(nc, in_maps, list(range(NCORES)), trace=trace)
    d_fake = np.zeros(NB, np.float64)
    for r in res.results:
        d_fake += _dpart_to_dfake(r["dpart"])
    return d_fake / SD, res


def _dilate(v):
    out = v.copy()
    out[:-1, :] |= v[1:, :]
    out[1:, :] |= v[:-1, :]
    out[:, :-1] |= v[:, 1:]
    out[:, 1:] |= v[:, :-1]
    return out


def _host_exact_maze_terms(noise, Wg):
    """Fallback (practically unreachable): exact wall/flood-fill computation."""
    solv = 0.0
    wall_total = 0
    for b0 in range(0, B, 64):
        x = noise[b0 : b0 + 64].astype(np.float32) @ Wg.astype(np.float32)
        fake = np.tanh(x).astype(np.float32)
        for j in range(fake.shape[0]):
            maze = fake[j].reshape(H, W)
            wall = maze == np.float32(1.0)
            nwall = int(wall.sum())
            wall_total += nwall
            pen = 0.0
            if float(wall.mean()) > 0.5:
                pen += 1.0
            if nwall >= 3:
                open_ = ~wall
                visited = np.zeros((H, W), bool)
                visited[1, 1] = True
                while True:
                    nv = visited | (_dilate(visited) & open_)
                    if not (nv & ~visited).any():
                        break
                    visited = nv
                wf = wall.astype(np.float32)
                wa = np.zeros((H, W), np.float32)
                wa[:-1, :] += wf[1:, :]
                wa[1:, :] += wf[:-1, :]
                wa[:, :-1] += wf[:, 1:]
                wa[:, 1:] += wf[:, :-1]
                pen += 0.1 * float((visited & (wa >= 3.0)).sum())
            solv += pen
    solv /= B
    cur = wall_total / float(B * H * W)
    return solv, cur


def kernel(**inputs) -> np.ndarray:
    noise = np.asarray(inputs["noise"], np.float32)
    Wg = np.asarray(inputs["Wg"], np.float32)
    Wd = np.asarray(inputs["Wd"], np.float32)
    p = float(np.asarray(inputs["maml_performance"]).reshape(-1)[0])
    cd = float(np.asarray(inputs["current_difficulty"]).reshape(-1)[0])

    d_fake, _ = run_device(noise, Wg, Wd)

    # g_loss = mean(softplus(-d_fake));  0.0 * sum(d_real) == 0 exactly.
    g_loss = float(np.mean(np.logaddexp(0.0, -d_fake)))

    # Wall existence bound: |x[b,n]| <= max_b||noise_b|| * max_n||Wg[:,n]||.
    rn = float(np.sqrt((noise.astype(np.float64) ** 2).sum(axis=1)).max())
    cn = float(np.sqrt((Wg.astype(np.float64) ** 2).sum(axis=0)).max())
    if rn * cn * 1.0001 < WALL_SAFE_BOUND:
        solv, cur = 0.0, 0.0
    else:  # pragma: no cover - requires |pre-tanh| ~ 28 sigma
        solv, cur = _host_exact_maze_terms(noise, Wg)

    w_s = 0.8 if p < 0.4 else (0.4 if p > 0.6 else 0.6)
    w_d = 0.05 if p < 0.4 else (0.2 if p > 0.6 else 0.1)
    difficulty = (cur - cd) ** 2
    loss = g_loss + w_s * solv + w_d * difficulty
    return np.array(loss, dtype=np.float32)
